# revision 2
# baseline (speedup 1.0000x reference)
"""HMNeRF Trainium2 kernel: big shared NeRF + 16 per-video small-NeRF experts.

Strategy (8 NeuronCores, SPMD data-parallel over samples):
- Host shards N=65536 samples into 8 contiguous slices of 8192, sorts each
  core's samples by vidid (stable), pads each expert group to a common
  capacity C, and inverse-permutes outputs at the end.
- Layout is feature-major: SBUF tiles are [features(partitions), samples].
- Big NeRF: 8-layer 256-wide MLP over 512-sample tiles. 256-wide layers are
  2 M-tiles x 2 K-chunks of 128. Skip concat at layer 4 is realized as
  extra accumulated K-chunk matmuls reading the input tile.
- mnerf experts are processed in pairs with block-diagonal [128,128]
  weights: expert a on partitions 0:64, expert b on 64:128.
- Biases are folded into matmuls where a spare "ones" partition row exists
  in the rhs (L0/L4 via x0 row 95, big-d via xdir row 27, mnerf W0 via xp
  row 126, mnerf d via dp row 54, mnerf rgb via d ones row produced by an
  extra Wd output column).
- All matmuls run in float32r (tf32-like, ~1e-4 rel err, full PE rate for
  free dims >= 256).
"""

import os
import numpy as np

import jax
from jax.sharding import Mesh, PartitionSpec, NamedSharding
from jax.experimental.shard_map import shard_map

import concourse.bass as bass
import concourse.mybir as mybir
import concourse.tile as tile
from concourse import bacc
from concourse.bass2jax import _bass_exec_p, install_neuronx_cc_hook, partition_id_tensor

# ---- problem constants (hardcoded; kernel.py must be self-contained) ----
N_TOT = 65536
V = 16
CODE = 32
CX, CD = 63, 27
D_BIG, W_BIG = 8, 256
W_SM = 64
SKIP = 4
NCORES = 8
TILE_N = 512

F32 = mybir.dt.float32
F32R = mybir.dt.float32r
AF = mybir.ActivationFunctionType
ALU = mybir.AluOpType

LAST_INFO = {}


# ---------------------------------------------------------------- layouts
class _Cols:
    def __init__(self):
        self.n = 0
        self.map = {}

    def alloc(self, key, width):
        self.map[key] = (self.n, width)
        self.n += width

    def __getitem__(self, key):
        return self.map[key]


def _big_nk(i):
    # K-chunk source list per layer: "x" = input tile (95+ones), ("h", j)
    if i == 0:
        return ["x"]
    if i == SKIP:
        return ["x", ("h", 0), ("h", 1)]
    return [("h", 0), ("h", 1)]


def _layout_big():
    wb = _Cols()
    for i in range(D_BIG):
        for m in range(2):
            for k in range(len(_big_nk(i))):
                wb.alloc(f"L{i}_m{m}_k{k}", 128)
    for m in range(2):
        for k in range(2):
            wb.alloc(f"Wf_m{m}_k{k}", 128)
    wb.alloc("Ws_k0", 1)
    wb.alloc("Ws_k1", 1)
    wb.alloc("Wd_k0", 128)
    wb.alloc("Wd_k1", 128)
    wb.alloc("Wdd", 128)   # rows 0:27 dir part, row 27 = bd (ones trick)
    wb.alloc("Wr", 3)
    return wb


def _layout_mn():
    wm = _Cols()
    for p in range(V // 2):
        wm.alloc(f"P{p}_W0", 128)   # block-diag, row 126 = [b0_a; b0_b]
        wm.alloc(f"P{p}_W1", 128)
        wm.alloc(f"P{p}_Wf", 128)
        wm.alloc(f"P{p}_Wdh", 65)   # M=65: col 64 zero
        wm.alloc(f"P{p}_Wdd", 65)   # rows 0:54 dir, row 54 = [bd;bd]; col64 row54=1
        wm.alloc(f"P{p}_Wr", 6)     # K=65: a rows 0:32 + row64=br_a (cols 0:3)
        wm.alloc(f"P{p}_Ws", 1)     # a rows 0:64, b rows 64:128
    return wm


def _layout_bias():
    bc = _Cols()
    for i in range(D_BIG):
        if i in (0, SKIP):
            continue  # folded into matmul
        for m in range(2):
            bc.alloc(f"b{i}_m{m}", 1)
    for m in range(2):
        bc.alloc(f"bf_m{m}", 1)
    bc.alloc("br", 1)  # [3,1]
    for p in range(V // 2):
        bc.alloc(f"P{p}_b1", 1)
        bc.alloc(f"P{p}_bf", 1)
    return bc


# ---------------------------------------------------------------- packing
def _pack_params(nerf, mnerf):
    """Returns (wb, wm, bias_arr, scalars) host numpy arrays."""
    wbL, wmL, bcL = _layout_big(), _layout_mn(), _layout_bias()
    wb = np.zeros((128, wbL.n), np.float32)
    wm = np.zeros((128, wmL.n), np.float32)
    bias = np.zeros((128, bcL.n), np.float32)

    def put(arr, layout, key, rows, block):
        c0, w = layout[key]
        block = np.asarray(block, np.float32)
        arr[rows[0]:rows[0] + block.shape[0], c0:c0 + block.shape[1]] = block

    # ---- big nerf ----
    in0 = CX + CODE  # 95
    for i in range(D_BIG):
        Wt = np.asarray(nerf[f"W{i}"], np.float32).T  # (in_dim, 256)
        b = np.asarray(nerf[f"b{i}"], np.float32)
        ks = _big_nk(i)
        for m in range(2):
            mc = slice(128 * m, 128 * (m + 1))
            for k, src in enumerate(ks):
                if src == "x":
                    blk = np.zeros((128, 128), np.float32)
                    blk[:in0] = Wt[:in0, mc]
                    blk[in0] = b[mc]  # ones row 95 of x0 carries the bias
                else:
                    j = src[1]
                    blk = Wt[in0 + 128 * j:in0 + 128 * (j + 1), mc] if i == SKIP \
                        else Wt[128 * j:128 * (j + 1), mc]
                key = f"L{i}_m{m}_k{k}"
                put(wb, wbL, key, (0,), blk)
    Wft = np.asarray(nerf["Wf"], np.float32).T
    for m in range(2):
        for k in range(2):
            put(wb, wbL, f"Wf_m{m}_k{k}", (0,),
                Wft[128 * k:128 * (k + 1), 128 * m:128 * (m + 1)])
    Wst = np.asarray(nerf["Ws"], np.float32).T  # (256,1)
    put(wb, wbL, "Ws_k0", (0,), Wst[0:128])
    put(wb, wbL, "Ws_k1", (0,), Wst[128:256])
    Wdt = np.asarray(nerf["Wd"], np.float32).T  # (283,128)
    put(wb, wbL, "Wd_k0", (0,), Wdt[0:128])
    put(wb, wbL, "Wd_k1", (0,), Wdt[128:256])
    blk = np.zeros((32, 128), np.float32)
    blk[0:27] = Wdt[256:283]
    blk[27] = np.asarray(nerf["bd"], np.float32)  # ones row 27 of xdir
    put(wb, wbL, "Wdd", (0,), blk)
    put(wb, wbL, "Wr", (0,), np.asarray(nerf["Wr"], np.float32).T)  # (128,3)

    # ---- mnerf (expert pairs) ----
    mW0 = np.asarray(mnerf["W0"], np.float32)
    mW1 = np.asarray(mnerf["W1"], np.float32)
    mWf = np.asarray(mnerf["Wf"], np.float32)
    mWd = np.asarray(mnerf["Wd"], np.float32)
    mWs = np.asarray(mnerf["Ws"], np.float32)
    mWr = np.asarray(mnerf["Wr"], np.float32)
    mb0 = np.asarray(mnerf["b0"], np.float32)
    mb1 = np.asarray(mnerf["b1"], np.float32)
    mbf = np.asarray(mnerf["bf"], np.float32)
    mbd = np.asarray(mnerf["bd"], np.float32)
    mbs = np.asarray(mnerf["bs"], np.float32)
    mbr = np.asarray(mnerf["br"], np.float32)
    for p in range(V // 2):
        a, b_ = 2 * p, 2 * p + 1
        blk = np.zeros((128, 128), np.float32)
        blk[0:63, 0:64] = mW0[a].T
        blk[63:126, 64:128] = mW0[b_].T
        blk[126, 0:64] = mb0[a]
        blk[126, 64:128] = mb0[b_]  # ones row 126 of xp
        put(wm, wmL, f"P{p}_W0", (0,), blk)
        for key, W in ((f"P{p}_W1", mW1), (f"P{p}_Wf", mWf)):
            blk = np.zeros((128, 128), np.float32)
            blk[0:64, 0:64] = W[a].T
            blk[64:128, 64:128] = W[b_].T
            put(wm, wmL, key, (0,), blk)
        # Wd: input [final(64); dir(27)] -> 32; M layout: a cols 0:32, b 32:64,
        # col 64 = ones output (via dp ones row 54)
        blk = np.zeros((128, 65), np.float32)
        blk[0:64, 0:32] = mWd[a].T[0:64]
        blk[64:128, 32:64] = mWd[b_].T[0:64]
        put(wm, wmL, f"P{p}_Wdh", (0,), blk)
        blk = np.zeros((64, 65), np.float32)
        blk[0:27, 0:32] = mWd[a].T[64:91]
        blk[27:54, 32:64] = mWd[b_].T[64:91]
        blk[54, 0:32] = mbd[a]
        blk[54, 32:64] = mbd[b_]
        blk[54, 64] = 1.0  # makes psum row 64 == 1 -> d ones row
        put(wm, wmL, f"P{p}_Wdd", (0,), blk)
        # Wr: K=65 over d[0:65] (d rows 0:32 = d_a, 32:64 = d_b, 64 = ones)
        blk = np.zeros((65, 6), np.float32)
        blk[0:32, 0:3] = mWr[a].T
        blk[64, 0:3] = mbr[a]
        blk[32:64, 3:6] = mWr[b_].T
        blk[64, 3:6] = mbr[b_]
        put(wm, wmL, f"P{p}_Wr", (0,), blk)
        blk = np.zeros((128, 1), np.float32)
        blk[0:64, 0] = mWs[a].T[:, 0]
        blk[64:128, 0] = mWs[b_].T[:, 0]
        put(wm, wmL, f"P{p}_Ws", (0,), blk)

    # ---- biases ----
    for i in range(D_BIG):
        if i in (0, SKIP):
            continue
        b = np.asarray(nerf[f"b{i}"], np.float32)
        for m in range(2):
            c0, _ = bcL[f"b{i}_m{m}"]
            bias[0:128, c0] = b[128 * m:128 * (m + 1)]
    bf = np.asarray(nerf["bf"], np.float32)
    for m in range(2):
        c0, _ = bcL[f"bf_m{m}"]
        bias[0:128, c0] = bf[128 * m:128 * (m + 1)]
    c0, _ = bcL["br"]
    bias[0:3, c0] = np.asarray(nerf["br"], np.float32)
    for p in range(V // 2):
        a, b_ = 2 * p, 2 * p + 1
        c0, _ = bcL[f"P{p}_b1"]
        bias[0:64, c0] = mb1[a]
        bias[64:128, c0] = mb1[b_]
        c0, _ = bcL[f"P{p}_bf"]
        bias[0:64, c0] = mbf[a]
        bias[64:128, c0] = mbf[b_]

    scalars = {
        "bs_big": float(np.asarray(nerf["bs"], np.float32)[0]),
        "bs_mn": [float(mbs[v, 0]) for v in range(V)],
    }
    return wb, wm, bias, scalars


# ------------------------------------------------------- engine balancer
class _Balance:
    """Greedy ACT/DVE load balancer for pointwise PSUM->SBUF ops."""

    def __init__(self, nc):
        self.nc = nc
        self.t = {"act": 0.0, "dve": 0.0}

    @staticmethod
    def _cost(engine, fd, psum_src):
        if engine == "act":
            return ((172 if psum_src else 224) + fd) / 1.2
        return ((120 if psum_src else 58) + fd) / 0.96

    def emit(self, out, in_, kind, bias=0.0, fd=None, psum_src=True,
             force=None):
        """kind in {relu, copy, sigmoid}; bias: AP | float."""
        nc = self.nc
        if fd is None:
            fd = in_.shape[-1]
        eng = force
        if eng is None:
            if kind == "sigmoid":
                eng = "act"
            else:
                ca = self.t["act"] + self._cost("act", fd, psum_src)
                cd = self.t["dve"] + self._cost("dve", fd, psum_src)
                eng = "act" if ca <= cd else "dve"
        self.t[eng] += self._cost(eng, fd, psum_src)
        if eng == "act":
            func = {"relu": AF.Relu, "copy": AF.Copy, "sigmoid": AF.Sigmoid}[kind]
            if kind == "copy" and not isinstance(bias, float):
                func = AF.Identity  # Copy rejects AP bias
            nc.scalar.activation(out, in_, func, bias=bias)
        else:
            if kind == "relu":
                nc.vector.tensor_scalar(out, in_, bias, 0.0, ALU.add, ALU.max)
            else:  # copy (+bias)
                nc.vector.tensor_scalar(out, in_, bias, None, ALU.add)

    def tt_add(self, out, a, b):
        self.t["dve"] += (max(a.shape[-1], 1) + 151) / 0.96
        self.nc.vector.tensor_add(out, a, b)


# ---------------------------------------------------------------- builder
def _build(C, scalars, iters=1):
    NP = V * C
    NPP = (V // 2) * C
    nchunk = -(-C // TILE_N)
    CH = C // nchunk
    nt = NP // TILE_N
    wbL, wmL, bcL = _layout_big(), _layout_mn(), _layout_bias()

    nc = bacc.Bacc("TRN2", target_bir_lowering=False, debug=False,
                   enable_asserts=False, num_devices=NCORES)
    d_x0 = nc.dram_tensor("x0", [128, NP], F32R, kind="ExternalInput").ap()
    d_xd = nc.dram_tensor("xd", [32, NP], F32R, kind="ExternalInput").ap()
    d_xp = nc.dram_tensor("xp", [128, NPP], F32R, kind="ExternalInput").ap()
    d_dp = nc.dram_tensor("dp", [64, NPP], F32R, kind="ExternalInput").ap()
    d_wb = nc.dram_tensor("wb", [128, wbL.n], F32R, kind="ExternalInput").ap()
    d_wm = nc.dram_tensor("wm", [128, wmL.n], F32R, kind="ExternalInput").ap()
    d_bias = nc.dram_tensor("bias", [128, bcL.n], F32, kind="ExternalInput").ap()
    d_orgb = nc.dram_tensor("out_rgb", [3, NP], F32, kind="ExternalOutput").ap()
    d_osig = nc.dram_tensor("out_sig", [1, NP], F32, kind="ExternalOutput").ap()

    with tile.TileContext(nc) as tc:
        with tc.tile_pool(name="const", bufs=1) as cp, \
             tc.tile_pool(name="inp", bufs=3) as ip, \
             tc.tile_pool(name="act", bufs=3) as ap_, \
             tc.tile_pool(name="psum", bufs=1, space="PSUM") as pp:

            wm_t = cp.tile([128, wmL.n], F32R)
            nc.sync.dma_start(wm_t[:], d_wm[:])
            wb_t = cp.tile([128, wbL.n], F32R)
            nc.sync.dma_start(wb_t[:], d_wb[:])
            bias_t = cp.tile([128, bcL.n], F32)
            nc.sync.dma_start(bias_t[:], d_bias[:])
            mrgb = cp.tile([3, NP], F32)
            msig = cp.tile([1, NP], F32)

            def wmc(key, rows=None):
                c0, w = wmL[key]
                return wm_t[0:rows, c0:c0 + w] if rows else wm_t[:, c0:c0 + w]

            def wbc(key, rows=None):
                c0, w = wbL[key]
                return wb_t[0:rows, c0:c0 + w] if rows else wb_t[:, c0:c0 + w]

            def bco(key, rows=128):
                c0, _ = bcL[key]
                return bias_t[0:rows, c0:c0 + 1]

            bal = _Balance(nc)
            MM = nc.tensor.matmul

            for _ in range(iters):
                # ================= mnerf phase =================
                for p in range(V // 2):
                    for u in range(nchunk):
                        cp0 = p * C + u * CH          # pair-stacked col base
                        ca = (2 * p) * C + u * CH     # grouped col base (a)
                        cb = (2 * p + 1) * C + u * CH
                        xp_t = ip.tile([128, CH], F32R, tag="xp")
                        nc.sync.dma_start(xp_t[:], d_xp[:, cp0:cp0 + CH])
                        dp_t = ip.tile([64, CH], F32R, tag="dp")
                        nc.sync.dma_start(dp_t[:], d_dp[:, cp0:cp0 + CH])

                        ps = pp.tile([128, CH], F32, tag="psA", bufs=2)
                        MM(ps[:], wmc(f"P{p}_W0"), xp_t[:], start=True, stop=True)
                        h0 = ap_.tile([128, CH], F32R, tag="mh0")
                        bal.emit(h0[:], ps[:], "relu")  # bias folded in W0
                        ps = pp.tile([128, CH], F32, tag="psB", bufs=2)
                        MM(ps[:], wmc(f"P{p}_W1"), h0[:], start=True, stop=True)
                        h1 = ap_.tile([128, CH], F32R, tag="mh1")
                        bal.emit(h1[:], ps[:], "relu", bias=bco(f"P{p}_b1"))
                        ps = pp.tile([128, CH], F32, tag="psA", bufs=2)
                        MM(ps[:], wmc(f"P{p}_Wf"), h1[:], start=True, stop=True)
                        fin = ap_.tile([128, CH], F32R, tag="mfin")
                        bal.emit(fin[:], ps[:], "copy", bias=bco(f"P{p}_bf"))
                        psd = pp.tile([128, CH], F32, tag="psB", bufs=2)
                        MM(psd[0:65, :], wmc(f"P{p}_Wdh"), fin[:],
                           start=True, stop=False)
                        MM(psd[0:65, :], wmc(f"P{p}_Wdd", rows=64), dp_t[:],
                           start=False, stop=True)
                        d_t = ap_.tile([128, CH], F32R, tag="md")
                        bal.emit(d_t[0:65, :], psd[0:65, :], "relu")  # row64->1
                        # heads (per expert so outputs land at partition 0)
                        pr = pp.tile([3, TILE_N], F32, tag="psR", bufs=2)
                        MM(pr[:, 0:CH], wmc(f"P{p}_Wr", rows=65)[:, 0:3],
                           d_t[0:65, :], start=True, stop=True)
                        bal.emit(mrgb[:, ca:ca + CH], pr[:, 0:CH], "sigmoid")
                        pr = pp.tile([3, TILE_N], F32, tag="psR", bufs=2)
                        MM(pr[:, 0:CH], wmc(f"P{p}_Wr", rows=65)[:, 3:6],
                           d_t[0:65, :], start=True, stop=True)
                        bal.emit(mrgb[:, cb:cb + CH], pr[:, 0:CH], "sigmoid")
                        psg = pp.tile([1, TILE_N], F32, tag="psS", bufs=2)
                        MM(psg[:, 0:CH], wmc(f"P{p}_Ws", rows=64),
                           h1[0:64, :], start=True, stop=True)
                        bal.emit(msig[:, ca:ca + CH], psg[:, 0:CH], "copy",
                                 bias=scalars["bs_mn"][2 * p])
                        psg = pp.tile([1, TILE_N], F32, tag="psS", bufs=2)
                        MM(psg[:, 0:CH], wm_t[64:128,
                           wmL[f"P{p}_Ws"][0]:wmL[f"P{p}_Ws"][0] + 1],
                           h1[64:128, :], start=True, stop=True)
                        bal.emit(msig[:, cb:cb + CH], psg[:, 0:CH], "copy",
                                 bias=scalars["bs_mn"][2 * p + 1])

                # ================= big nerf phase =================
                for t in range(nt):
                    s = t * TILE_N
                    x0_t = ip.tile([128, TILE_N], F32R, tag="x0")
                    nc.sync.dma_start(x0_t[:], d_x0[:, s:s + TILE_N])
                    xd_t = ip.tile([32, TILE_N], F32R, tag="xdr")
                    nc.sync.dma_start(xd_t[:], d_xd[:, s:s + TILE_N])

                    h = [None, None]
                    for i in range(D_BIG):
                        nh = []
                        for m in range(2):
                            ps = pp.tile([128, TILE_N], F32,
                                         tag=f"ps{'AB'[m]}", bufs=2)
                            ks = _big_nk(i)
                            for k, src in enumerate(ks):
                                rhs = x0_t[:] if src == "x" else h[src[1]][:]
                                MM(ps[:], wbc(f"L{i}_m{m}_k{k}"), rhs,
                                   start=(k == 0), stop=(k == len(ks) - 1))
                            ht = ap_.tile([128, TILE_N], F32R, tag=f"h{m}")
                            if i in (0, SKIP):
                                bal.emit(ht[:], ps[:], "relu")
                            else:
                                bal.emit(ht[:], ps[:], "relu",
                                         bias=bco(f"b{i}_m{m}"))
                            nh.append(ht)
                        h = nh
                    # heads
                    fins = []
                    for m in range(2):
                        ps = pp.tile([128, TILE_N], F32,
                                     tag=f"ps{'AB'[m]}", bufs=2)
                        for k in range(2):
                            MM(ps[:], wbc(f"Wf_m{m}_k{k}"), h[k][:],
                               start=(k == 0), stop=(k == 1))
                        ft = ap_.tile([128, TILE_N], F32R, tag=f"fin{m}")
                        bal.emit(ft[:], ps[:], "copy", bias=bco(f"bf_m{m}"))
                        fins.append(ft)
                    pss = pp.tile([1, TILE_N], F32, tag="psS", bufs=2)
                    MM(pss[:], wbc("Ws_k0"), h[0][:], start=True, stop=False)
                    MM(pss[:], wbc("Ws_k1"), h[1][:], start=False, stop=True)
                    tsig = ap_.tile([1, TILE_N], F32, tag="tsig")
                    bal.emit(tsig[:], pss[:], "copy", bias=scalars["bs_big"])
                    psd = pp.tile([128, TILE_N], F32, tag="psA", bufs=2)
                    MM(psd[:], wbc("Wd_k0"), fins[0][:], start=True, stop=False)
                    MM(psd[:], wbc("Wd_k1"), fins[1][:], start=False, stop=False)
                    MM(psd[:], wbc("Wdd", rows=32), xd_t[:],
                       start=False, stop=True)
                    dbig = ap_.tile([128, TILE_N], F32R, tag="dbig")
                    bal.emit(dbig[:], psd[:], "relu")  # bd via xd ones row
                    psr = pp.tile([3, TILE_N], F32, tag="psR", bufs=2)
                    MM(psr[:], wbc("Wr", rows=128), dbig[:],
                       start=True, stop=True)
                    rgb2 = ap_.tile([3, TILE_N], F32, tag="rgb2")
                    bal.emit(rgb2[:], psr[:], "sigmoid", bias=bco("br", rows=3))
                    # combine with mnerf + final nonlinearity
                    tmp = ap_.tile([3, TILE_N], F32, tag="tmpr")
                    bal.tt_add(tmp[:], rgb2[:], mrgb[:, s:s + TILE_N])
                    orgb_t = ap_.tile([3, TILE_N], F32, tag="orgb")
                    bal.emit(orgb_t[:], tmp[:], "sigmoid", psum_src=False)
                    nc.sync.dma_start(d_orgb[:, s:s + TILE_N], orgb_t[:])
                    osig_t = ap_.tile([1, TILE_N], F32, tag="osig")
                    bal.tt_add(osig_t[:], tsig[:], msig[:, s:s + TILE_N])
                    nc.sync.dma_start(d_osig[:, s:s + TILE_N], osig_t[:])

    nc.compile()
    return nc


# ---------------------------------------------------------------- runner
class _Handle:
    """Compiled SPMD executable with a persistent jitted callable."""

    def __init__(self, nc):
        install_neuronx_cc_hook()
        self.nc = nc
        pname = nc.partition_id_tensor.name if nc.partition_id_tensor else None
        in_names, out_names, out_avals = [], [], []
        for alloc in nc.m.functions[0].allocations:
            if not isinstance(alloc, mybir.MemoryLocationSet):
                continue
            name = alloc.memorylocations[0].name
            if alloc.kind == "ExternalInput":
                if name != pname:
                    in_names.append(name)
            elif alloc.kind == "ExternalOutput":
                out_names.append(name)
                out_avals.append(jax.core.ShapedArray(
                    tuple(alloc.tensor_shape), mybir.dt.np(alloc.dtype)))
        self.in_names, self.out_names, self.out_avals = \
            in_names, out_names, out_avals
        all_names = in_names + out_names + ([pname] if pname else [])

        def _body(*args):
            operands = list(args)
            if pname is not None:
                operands.append(partition_id_tensor())
            return tuple(_bass_exec_p.bind(
                *operands, out_avals=tuple(out_avals),
                in_names=tuple(all_names), out_names=tuple(out_names),
                lowering_input_output_aliases=(),
                sim_require_finite=False, sim_require_nnan=False, nc=nc))

        devices = jax.devices()[:NCORES]
        self.mesh = Mesh(np.asarray(devices), ("core",))
        nin = len(in_names) + len(out_names)
        self.f = jax.jit(shard_map(
            _body, mesh=self.mesh,
            in_specs=(PartitionSpec("core"),) * nin,
            out_specs=(PartitionSpec("core"),) * len(out_names),
            check_rep=False), keep_unused=True)

    def put_args(self, in_maps):
        concat = [np.concatenate([np.asarray(m[nm]) for m in in_maps], axis=0)
                  for nm in self.in_names]
        zeros = [np.zeros((NCORES * a.shape[0], *a.shape[1:]), a.dtype)
                 for a in self.out_avals]
        sh = NamedSharding(self.mesh, PartitionSpec("core"))
        return [jax.device_put(a, sh) for a in concat + zeros]

    def run(self, args):
        outs = self.f(*args)
        jax.block_until_ready(outs)
        return {name: np.asarray(outs[i]).reshape(
                    NCORES, *self.out_avals[i].shape)
                for i, name in enumerate(self.out_names)}


_CACHE = {}


def _get_handle(C, scalars, iters=1):
    key = (C, iters, scalars["bs_big"], tuple(scalars["bs_mn"]))
    if key not in _CACHE:
        _CACHE[key] = _Handle(_build(C, scalars, iters=iters))
    return _CACHE[key]


# ------------------------------------------------------------- host prep
def _round_up(x, m):
    return -(-x // m) * m


def _prep_inputs(x, vidid, nerf, mnerf):
    x = np.asarray(x, np.float32)
    vid = np.asarray(vidid).astype(np.int64)
    per = x.shape[0] // NCORES
    cores = []
    maxc = 0
    for c in range(NCORES):
        vc = vid[c * per:(c + 1) * per]
        order = np.argsort(vc, kind="stable")
        counts = np.bincount(vc, minlength=V)[:V]
        maxc = max(maxc, int(counts.max()))
        cores.append((order, counts))
    C = max(_round_up(maxc, 64), 256)
    nchunk = -(-C // TILE_N)
    C = _round_up(C, 32 * nchunk)
    NP, NPP = V * C, (V // 2) * C

    code = np.asarray(nerf["code"], np.float32)  # (V, 32)
    wb, wm, bias, scalars = _pack_params(nerf, mnerf)

    in_maps, metas = [], []
    for c in range(NCORES):
        order, counts = cores[c]
        xs = x[c * per:(c + 1) * per][order]
        xyzT = np.ascontiguousarray(xs[:, :CX].T)   # (63, per)
        dirT = np.ascontiguousarray(xs[:, CX:].T)   # (27, per)
        x0 = np.zeros((128, NP), np.float32)
        xd = np.zeros((32, NP), np.float32)
        xp = np.zeros((128, NPP), np.float32)
        dp = np.zeros((64, NPP), np.float32)
        x0[95] = 1.0
        xd[27] = 1.0
        xp[126] = 1.0
        dp[54] = 1.0
        cum = np.concatenate([[0], np.cumsum(counts)])
        for v in range(V):
            sl = slice(int(cum[v]), int(cum[v + 1]))
            n = int(counts[v])
            g = v * C
            x0[0:CX, g:g + n] = xyzT[:, sl]
            x0[CX:95, g:g + n] = code[v][:, None]
            xd[0:CD, g:g + n] = dirT[:, sl]
            p, e = divmod(v, 2)
            pg = p * C
            xp[63 * e:63 * e + CX, pg:pg + n] = xyzT[:, sl]
            dp[27 * e:27 * e + CD, pg:pg + n] = dirT[:, sl]
        in_maps.append({"x0": x0, "xd": xd, "xp": xp, "dp": dp,
                        "wb": wb, "wm": wm, "bias": bias})
        metas.append((order, counts, cum))
    return C, scalars, in_maps, metas, per


def _unpack(res, metas, per, C):
    out = np.empty((N_TOT, 4), np.float32)
    for c in range(NCORES):
        order, counts, cum = metas[c]
        rgb = res["out_rgb"][c]   # (3, NP)
        sig = res["out_sig"][c]   # (1, NP)
        oc = np.empty((per, 4), np.float32)
        for v in range(V):
            n = int(counts[v])
            g = v * C
            idx = order[int(cum[v]):int(cum[v + 1])]
            oc[idx, 0:3] = rgb[:, g:g + n].T
            oc[idx, 3] = sig[0, g:g + n]
        out[c * per:(c + 1) * per] = oc
    return out


# ----------------------------------------------------------------- entry
def kernel(x, vidid, nerf_params, mnerf_params):
    C, scalars, in_maps, metas, per = _prep_inputs(
        x, vidid, nerf_params, mnerf_params)
    h = _get_handle(C, scalars, iters=int(os.environ.get("KERNEL_ITERS", "1")))
    args = h.put_args(in_maps)
    res = h.run(args)
    LAST_INFO["handle"] = h
    LAST_INFO["args"] = args
    LAST_INFO["C"] = C
    return _unpack(res, metas, per, C)
